# revision 43
# baseline (speedup 1.0000x reference)
"""Multi-head attention (B=2, S=2048, D=1024, H=16) on 8 TRN2 NeuronCores.

Sharding: core c handles batch b = c//4 and head-group g = c%4 (4 heads each).
Each core computes its heads' attention and a partial output projection
(row-parallel W_o); the host sums the 4 partials per batch and adds b_o.

Device-side layout trick: everything runs in the "transposed world".
The host passes x[b].T and mask.T, so the QK projection produces Q^T/K^T
directly (no on-device transposes), scores are computed as S^T = K·Q^T with
keys on partitions, softmax denominators come from a ones-column appended to
V, and the output projection produces out^T which the host transposes back.

Mask specialization: the host classifies each [128 x 512] tile of mask^T as
fully-masked (<= -1e8 everywhere: the whole tile contributes exp() == 0 and
is skipped), exactly zero (no mask add needed), or partial (mask added on
DVE). The program is built (and cached) per observed pattern, so any additive
mask is handled correctly; a causal mask skips ~45% of the attention work.
"""

import numpy as np

import concourse.bass as bass  # noqa: F401
import concourse.mybir as mybir
import concourse.tile as tile
from concourse import bacc
from concourse.bass import ds, ts
from concourse.bass_utils import run_bass_kernel_spmd

B, S, D, H = 2, 2048, 1024, 16
HD = D // H  # 64
HPC = 4      # heads per core
NCORES = 8
F32R = mybir.dt.float32r
F32 = mybir.dt.float32
AF = mybir.ActivationFunctionType
ADD = mybir.AluOpType.add
MULT = mybir.AluOpType.mult

SKIP, FULL, PART = 0, 1, 2

_CACHE = {}


def _classify_mask(maskT):
    """pattern[hf][kb][nn] for [128, 512] tiles of mask^T (k major, q minor)."""
    pat = []
    for hf in range(2):
        rows = []
        for kb in range(16):
            ents = []
            for nn in range(2):
                blk = maskT[kb * 128 : (kb + 1) * 128,
                            hf * 1024 + nn * 512 : hf * 1024 + (nn + 1) * 512]
                if np.all(blk <= -1e8):
                    ents.append(SKIP)
                elif np.all(blk == 0.0):
                    ents.append(FULL)
                else:
                    ents.append(PART)
            rows.append(tuple(ents))
        pat.append(tuple(rows))
    return tuple(tuple(r) for r in pat)


def _build(pattern):
    nc = bacc.Bacc(None, target_bir_lowering=False, debug=False)
    xT = nc.dram_tensor("xT", [D, S], F32R, kind="ExternalInput")
    wqk = nc.dram_tensor("wqk", [D, 512], F32R, kind="ExternalInput")
    bqk = nc.dram_tensor("bqk", [128, 4], F32R, kind="ExternalInput")
    wv = nc.dram_tensor("wv", [D, 256], F32R, kind="ExternalInput")
    bv = nc.dram_tensor("bv", [1, 256], F32R, kind="ExternalInput")
    wo = nc.dram_tensor("wo", [256, D], F32R, kind="ExternalInput")
    maskT = nc.dram_tensor("maskT", [S, S], F32R, kind="ExternalInput")
    outT = nc.dram_tensor("outT", [D, S], F32, kind="ExternalOutput")

    with tile.TileContext(nc) as tc:
        with (
            tc.tile_pool(name="big", bufs=1) as bigp,
            tc.tile_pool(name="wqkp", bufs=1) as wqkp,
            tc.tile_pool(name="wvp", bufs=1) as wvp,
            tc.tile_pool(name="wop", bufs=1) as wop,
            tc.tile_pool(name="qkp", bufs=1) as qkp,
            tc.tile_pool(name="vp", bufs=1) as vp,
            tc.tile_pool(name="valsp", bufs=1) as valsp,
            tc.tile_pool(name="attnp", bufs=3) as attnp,
            tc.tile_pool(name="maskp", bufs=1) as maskp,
            tc.tile_pool(name="smallp", bufs=1) as smallp,
            tc.tile_pool(name="constp", bufs=1) as constp,
        ):
            ones_t = constp.tile([1, 128], F32R)
            nc.gpsimd.memset(ones_t[:].bitcast(F32), 1.0)
            b_sb = constp.tile([128, 4], F32R)
            nc.sync.dma_start(b_sb[:], bqk[:])
            bv_sb = constp.tile([1, 256], F32R)
            nc.sync.dma_start(bv_sb[:], bv[:])

            # s-half-major DMA: all of x^T's first s-half lands in ~half the
            # time, so half-0 projections and the first q-half's attention
            # can start much earlier
            wqk_sb = wqkp.tile([128, 8, 512], F32R, tag="wqkslot")
            xt_sb = bigp.tile([128, 8, S], F32R, tag="big")
            wv_sb = wvp.tile([128, 8, 256], F32R)
            for dc in range(8):
                nc.sync.dma_start(wqk_sb[:, dc, :], wqk[ds(dc * 128, 128), :])
                nc.sync.dma_start(
                    xt_sb[:, dc, ds(0, 1024)], xT[ds(dc * 128, 128), ds(0, 1024)]
                )
                nc.sync.dma_start(wv_sb[:, dc, :], wv[ds(dc * 128, 128), :])

            # qk_sb rows (partition+chunk) = projected qkv column:
            # chunk 0: q of heads 0,1; chunk 1: q of heads 2,3;
            # chunk 2: k of heads 0,1; chunk 3: k of heads 2,3.
            qk_sb = qkp.tile([128, 4, S], F32R)
            # v_sb[s%128, s//128, h, 0:64] = V; [..., 64] = 1.0 (denominator col)
            v_sb = vp.tile([128, 16, HPC, 65], F32R)
            
            nc.gpsimd.memset(v_sb[:, :, :, 64:65].bitcast(F32), 1.0)

            # mask classification; the first-processed half's mask tiles are
            # DMA'd inside the front stream so its PART adds never stall
            def n_full(hf):
                return sum(c == FULL for kb in pattern[hf] for c in kb)

            hf_order = sorted(range(2), key=lambda hf: n_full(hf))
            part_blocks, slots, mask_tiles = {}, {}, {}
            for hf_i, hf in enumerate(hf_order):
                pb = [
                    (kb, nn)
                    for kb in range(16) for nn in range(2)
                    if pattern[hf][kb][nn] == PART
                ]
                part_blocks[hf] = pb
                slots[hf] = {blk: i for i, blk in enumerate(pb)}
                if len(pb) == 0:
                    mask_tiles[hf] = None
                elif len(pb) <= 8:
                    if hf_i == 0:
                        mask_tiles[hf] = maskp.tile(
                            [128, 8, 512], F32R, tag="mask", name=f"mask{hf}")
                    else:
                        mask_tiles[hf] = wqkp.tile(
                            [128, 8, 512], F32R, tag="wqkslot", name=f"mask{hf}")
                else:
                    mask_tiles[hf] = bigp.tile(
                        [128, 32, 512], F32R, tag="big", name=f"mask{hf}")
            hf_first = hf_order[0]
            if mask_tiles.get(hf_first) is not None and len(part_blocks[hf_first]) <= 8:
                for (kb, nn), i in slots[hf_first].items():
                    nc.sync.dma_start(
                        mask_tiles[hf_first][:, i, :],
                        maskT[ds(kb * 128, 128),
                              ds(hf_first * 1024 + nn * 512, 512)],
                    )
            # second s-half of x^T + wo
            for dc in range(8):
                nc.sync.dma_start(
                    xt_sb[:, dc, ds(1024, 1024)],
                    xT[ds(dc * 128, 128), ds(1024, 1024)],
                )
            wo_sb = wop.tile([128, 2, D], F32R)
            nc.sync.dma_start(wo_sb[:], wo[:].rearrange("(kc p) d -> p kc d", p=128))

            # ---- projections: QK chunks 0,2 first (unblocks heads 0/1),
            # then V, then QK chunks 1,3 (heads 2/3). qkT[c,s] = W^T x^T.
            def qk_chunk_half(pool, cc, sh):
                ps = pool.tile([128, 1024], F32, tag="psqk", name=f"qkps{cc}{sh}")
                for dc in range(8):
                    lhsT = wqk_sb[:, dc, ts(cc, 128)]
                    for nn in range(2):
                        nc.tensor.matmul(
                            ps[:, ts(nn, 512)], lhsT,
                            xt_sb[:, dc, ds(sh * 1024 + nn * 512, 512)],
                            start=(dc == 0), stop=(dc == 7),
                        )
                nc.scalar.activation(
                    qk_sb[:, cc, ds(sh * 1024, 1024)], ps[:], AF.Identity,
                    bias=b_sb[:, cc : cc + 1],
                )

            def v_blocks(pool, rng):
                for sb_i in rng:
                    ps = pool.tile([128, 256], F32, tag="psv", name=f"vps{sb_i}")
                    for dc in range(8):
                        nc.tensor.matmul(
                            ps[:], xt_sb[:, dc, ts(sb_i, 128)], wv_sb[:, dc, :],
                            start=(dc == 0), stop=False,
                        )
                    # += ones[s] * bv  (rank-1 bias add)
                    nc.tensor.matmul(ps[:], ones_t[:], bv_sb[:], start=False, stop=True)
                    nc.vector.tensor_copy(
                        v_sb[:, sb_i, :, 0:64],
                        ps[:].rearrange("p (h e) -> p h e", h=HPC),
                    )

            # half-0 projections first (q-half-0 attention unblocks early),
            # then half-1
            with (
                tc.tile_pool(name="psqk", bufs=2, space="PSUM") as psqk,
                tc.tile_pool(name="psv", bufs=2, space="PSUM") as psv,
            ):
                for cc in (0, 2, 1, 3):
                    qk_chunk_half(psqk, cc, 0)
                v_blocks(psv, range(8))
                for cc in (0, 2, 1, 3):
                    qk_chunk_half(psqk, cc, 1)
                v_blocks(psv, range(8, 16))

            # ---- attention, in [k, q] layout, q processed in two halves ----
            # (mask tiles + first-half mask DMAs hoisted into the front stream)
            with (
                tc.tile_pool(name="pssc", bufs=3, space="PSUM") as pssc,
                tc.tile_pool(name="psav", bufs=1, space="PSUM") as psav,
            ):
                def outproj(vals_t, qs_):
                    for ob in range(8):
                        ps = pssc.tile([128, 1024], F32, tag="pssc")
                        for kc in range(2):
                            lhsT = wo_sb[:, kc, ts(ob, 128)]
                            for nn in range(2):
                                nc.tensor.matmul(
                                    ps[:, ts(nn, 512)], lhsT,
                                    vals_t[:, kc, ts(nn, 512)],
                                    start=(kc == 0), stop=(kc == 1),
                                )
                        oev = attnp.tile([128, 1024], F32, tag="attn")
                        nc.scalar.activation(oev[:], ps[:], AF.Copy)
                        nc.sync.dma_start(
                            outT[ds(ob * 128, 128), ds(qs_, 1024)], oev[:]
                        )

                pending = None
                for hf_i, hf in enumerate(hf_order):
                    qs = hf * 1024
                    slot = slots[hf]
                    mask_sb = mask_tiles[hf]
                    if mask_sb is not None and (
                        hf_i > 0 or len(part_blocks[hf]) > 8
                    ):
                        for (kb, nn), i in slot.items():
                            nc.sync.dma_start(
                                mask_sb[:, i, :],
                                maskT[ds(kb * 128, 128), ds(qs + nn * 512, 512)],
                            )
                    vals_sb = valsp.tile(
                        [128, 2, 1024], F32R, tag="vals", name=f"vals{hf}")
                    kb_order = sorted(
                        (kb for kb in range(16) if pattern[hf][kb] != (SKIP, SKIP)),
                        key=lambda kb: (PART in pattern[hf][kb], kb),
                    )
                    for h in range(HPC):
                        off = 64 * (h % 2)
                        qt = qk_sb[off : off + 64, h // 2, :]
                        kt = qk_sb[off : off + 64, 2 + h // 2, :]
                        act_kbs = [
                            [kb for kb in kb_order if pattern[hf][kb][nn] != SKIP]
                            for nn in range(2)
                        ]
                        ps_av = psav.tile([65, 1024], F32, tag="psav")
                        for kb in kb_order:
                            cls = pattern[hf][kb]
                            ps_sc = pssc.tile([128, 1024], F32, tag="pssc")
                            lhsT = kt[:, ts(kb, 128)]
                            at = attnp.tile([128, 1024], F32R, tag="attn")
                            for nn in range(2):
                                if cls[nn] == SKIP:
                                    continue
                                nc.tensor.matmul(
                                    ps_sc[:, ts(nn, 512)], lhsT,
                                    qt[:, ds(qs + nn * 512, 512)],
                                    start=True, stop=True,
                                )
                            if cls == (FULL, FULL):
                                nc.scalar.activation(at[:], ps_sc[:], AF.Exp)
                            else:
                                for nn in range(2):
                                    if cls[nn] == SKIP:
                                        continue
                                    if cls[nn] == PART:
                                        nc.vector.tensor_tensor(
                                            at[:, ts(nn, 512)], ps_sc[:, ts(nn, 512)],
                                            mask_sb[:, slot[(kb, nn)], :], ADD,
                                        )
                                        nc.scalar.activation(
                                            at[:, ts(nn, 512)], at[:, ts(nn, 512)],
                                            AF.Exp,
                                        )
                                    else:
                                        nc.scalar.activation(
                                            at[:, ts(nn, 512)], ps_sc[:, ts(nn, 512)],
                                            AF.Exp,
                                        )
                            for nn in range(2):
                                if cls[nn] == SKIP:
                                    continue
                                nc.tensor.matmul(
                                    ps_av[:, ts(nn, 512)], v_sb[:, kb, h, :],
                                    at[:, ts(nn, 512)],
                                    start=(kb == act_kbs[nn][0]),
                                    stop=(kb == act_kbs[nn][-1]),
                                )
                        # normalize: vals = av[0:64] * (1 / av[64])
                        recip = smallp.tile([1, 1024], F32R, tag="recip")
                        with nc.allow_low_precision(
                            reason="float32r has fp32 bits; only PE matmul mode differs"
                        ):
                            nc.vector.reciprocal(recip[:], ps_av[64:65, :])
                        bc_sb = smallp.tile([64, 1024], F32R, tag="bc")
                        nc.gpsimd.partition_broadcast(bc_sb[:], recip[:])
                        nc.vector.tensor_tensor(
                            vals_sb[off : off + 64, h // 2, :],
                            ps_av[0:64, :], bc_sb[:], MULT,
                        )
                        if pending is not None and hf_i == 1 and h == 0:
                            outproj(*pending)
                            pending = None

                    # partial output projection for this q-half; the first
                    # half's is deferred into the second half's first head
                    # sweep so it fills gaps instead of competing at the
                    # transition (execution follows deps, not emission order)
                    if hf_i == 0 and len(hf_order) > 1:
                        pending = (vals_sb, qs)
                    else:
                        outproj(vals_sb, qs)
                if pending is not None:
                    outproj(*pending)
                    pending = None

    nc.compile()
    return nc


def _prep_inputs(x, mask, W_qkv, b_qkv, W_o, b_o):
    """Host-side sharding/layout prep: slices, transposes, 1/sqrt(HD) folding."""
    scale = np.float32(1.0 / np.sqrt(HD))
    xT = [np.ascontiguousarray(x[b].T) for b in range(B)]
    maskT = np.ascontiguousarray(mask.T)
    in_maps = []
    for c in range(NCORES):
        b, g = divmod(c, HPC)
        heads = [HPC * g + i for i in range(HPC)]
        qcols = np.concatenate(
            [W_qkv[:, 192 * h : 192 * h + 64] for h in heads], axis=1) * scale
        kcols = np.concatenate(
            [W_qkv[:, 192 * h + 64 : 192 * h + 128] for h in heads], axis=1)
        wqk = np.ascontiguousarray(np.concatenate([qcols, kcols], axis=1))
        bq = np.concatenate([b_qkv[192 * h : 192 * h + 64] for h in heads]) * scale
        bk = np.concatenate([b_qkv[192 * h + 64 : 192 * h + 128] for h in heads])
        bqk_t = np.ascontiguousarray(
            np.concatenate([bq, bk]).reshape(4, 128).T)
        wv = np.ascontiguousarray(np.concatenate(
            [W_qkv[:, 192 * h + 128 : 192 * h + 192] for h in heads], axis=1))
        bv = np.ascontiguousarray(np.concatenate(
            [b_qkv[192 * h + 128 : 192 * h + 192] for h in heads])[None, :])
        wo = np.ascontiguousarray(W_o[256 * g : 256 * (g + 1), :])
        in_maps.append({
            "xT": xT[b], "wqk": wqk, "bqk": bqk_t, "wv": wv, "bv": bv,
            "wo": wo, "maskT": maskT,
        })
    return in_maps


def kernel(x, mask, W_qkv, b_qkv, W_o, b_o):
    x = np.asarray(x, dtype=np.float32)
    mask = np.asarray(mask, dtype=np.float32)
    W_qkv = np.asarray(W_qkv, dtype=np.float32)
    b_qkv = np.asarray(b_qkv, dtype=np.float32)
    W_o = np.asarray(W_o, dtype=np.float32)
    b_o = np.asarray(b_o, dtype=np.float32)

    pattern = _classify_mask(np.ascontiguousarray(mask.T))
    key = ("nc", pattern)
    if key not in _CACHE:
        _CACHE[key] = _build(pattern)
    nc = _CACHE[key]
    _CACHE["nc"] = nc

    in_maps = _prep_inputs(x, mask, W_qkv, b_qkv, W_o, b_o)
    res = run_bass_kernel_spmd(nc, in_maps, core_ids=list(range(NCORES)))
    _CACHE["last_result"] = res

    out = np.empty((B, S, D), dtype=np.float32)
    for b in range(B):
        acc = res.results[HPC * b]["outT"].astype(np.float32)
        for g in range(1, HPC):
            acc = acc + res.results[HPC * b + g]["outT"]
        out[b] = acc.T + b_o
    return out


# revision 46
# speedup vs baseline: 1.0044x; 1.0044x over previous
"""Multi-head attention (B=2, S=2048, D=1024, H=16) on 8 TRN2 NeuronCores.

Sharding: core c handles batch b = c//4 and head-group g = c%4 (4 heads each).
Each core computes its heads' attention and a partial output projection
(row-parallel W_o); the host sums the 4 partials per batch and adds b_o.

Device-side layout trick: everything runs in the "transposed world".
The host passes x[b].T and mask.T, so the QK projection produces Q^T/K^T
directly (no on-device transposes), scores are computed as S^T = K·Q^T with
keys on partitions, softmax denominators come from a ones-column appended to
V, and the output projection produces out^T which the host transposes back.

Mask specialization: the host classifies each [128 x 512] tile of mask^T as
fully-masked (<= -1e8 everywhere: the whole tile contributes exp() == 0 and
is skipped), exactly zero (no mask add needed), or partial (mask added on
DVE). The program is built (and cached) per observed pattern, so any additive
mask is handled correctly; a causal mask skips ~45% of the attention work.
"""

import numpy as np

import concourse.bass as bass  # noqa: F401
import concourse.mybir as mybir
import concourse.tile as tile
from concourse import bacc
from concourse.bass import ds, ts
from concourse.bass_utils import run_bass_kernel_spmd

B, S, D, H = 2, 2048, 1024, 16
HD = D // H  # 64
HPC = 4      # heads per core
NCORES = 8
F32R = mybir.dt.float32r
F32 = mybir.dt.float32
AF = mybir.ActivationFunctionType
ADD = mybir.AluOpType.add
MULT = mybir.AluOpType.mult

SKIP, FULL, PART = 0, 1, 2

_CACHE = {}


def _classify_mask(maskT):
    """pattern[hf][kb][nn] for [128, 512] tiles of mask^T (k major, q minor)."""
    pat = []
    for hf in range(2):
        rows = []
        for kb in range(16):
            ents = []
            for nn in range(2):
                blk = maskT[kb * 128 : (kb + 1) * 128,
                            hf * 1024 + nn * 512 : hf * 1024 + (nn + 1) * 512]
                if np.all(blk <= -1e8):
                    ents.append(SKIP)
                elif np.all(blk == 0.0):
                    ents.append(FULL)
                else:
                    ents.append(PART)
            rows.append(tuple(ents))
        pat.append(tuple(rows))
    return tuple(tuple(r) for r in pat)


def _build(pattern):
    nc = bacc.Bacc(None, target_bir_lowering=False, debug=False)
    xT = nc.dram_tensor("xT", [D, S], F32R, kind="ExternalInput")
    wqk = nc.dram_tensor("wqk", [D, 512], F32R, kind="ExternalInput")
    bqk = nc.dram_tensor("bqk", [128, 4], F32R, kind="ExternalInput")
    wv = nc.dram_tensor("wv", [D, 256], F32R, kind="ExternalInput")
    bv = nc.dram_tensor("bv", [1, 256], F32R, kind="ExternalInput")
    wo = nc.dram_tensor("wo", [256, D], F32R, kind="ExternalInput")
    maskT = nc.dram_tensor("maskT", [S, S], F32R, kind="ExternalInput")
    outT = nc.dram_tensor("outT", [D, S], F32, kind="ExternalOutput")

    with tile.TileContext(nc) as tc:
        with (
            tc.tile_pool(name="big", bufs=1) as bigp,
            tc.tile_pool(name="wqkp", bufs=1) as wqkp,
            tc.tile_pool(name="wvp", bufs=1) as wvp,
            tc.tile_pool(name="wop", bufs=1) as wop,
            tc.tile_pool(name="qkp", bufs=1) as qkp,
            tc.tile_pool(name="vp", bufs=1) as vp,
            tc.tile_pool(name="valsp", bufs=1) as valsp,
            tc.tile_pool(name="attnp", bufs=3) as attnp,
            tc.tile_pool(name="maskp", bufs=1) as maskp,
            tc.tile_pool(name="smallp", bufs=1) as smallp,
            tc.tile_pool(name="constp", bufs=1) as constp,
        ):
            ones_t = constp.tile([1, 128], F32R)
            nc.gpsimd.memset(ones_t[:].bitcast(F32), 1.0)
            b_sb = constp.tile([128, 4], F32R)
            nc.sync.dma_start(b_sb[:], bqk[:])
            bv_sb = constp.tile([1, 256], F32R)
            nc.sync.dma_start(bv_sb[:], bv[:])

            # s-half-major DMA: all of x^T's first s-half lands in ~half the
            # time, so half-0 projections and the first q-half's attention
            # can start much earlier
            wqk_sb = wqkp.tile([128, 8, 512], F32R, tag="wqkslot")
            xt_sb = bigp.tile([128, 8, S], F32R, tag="big")
            wv_sb = wvp.tile([128, 8, 256], F32R)
            for dc in range(8):
                nc.sync.dma_start(wqk_sb[:, dc, :], wqk[ds(dc * 128, 128), :])
                nc.sync.dma_start(
                    xt_sb[:, dc, ds(0, 1024)], xT[ds(dc * 128, 128), ds(0, 1024)]
                )
            nc.sync.dma_start(wv_sb[:], wv[:].rearrange("(dc p) c -> p dc c", p=128))

            # qk_sb rows (partition+chunk) = projected qkv column:
            # chunk 0: q of heads 0,1; chunk 1: q of heads 2,3;
            # chunk 2: k of heads 0,1; chunk 3: k of heads 2,3.
            qk_sb = qkp.tile([128, 4, S], F32R)
            # v_sb[s%128, s//128, h, 0:64] = V; [..., 64] = 1.0 (denominator col)
            v_sb = vp.tile([128, 16, HPC, 65], F32R)
            
            nc.gpsimd.memset(v_sb[:, :, :, 64:65].bitcast(F32), 1.0)

            # mask classification; the first-processed half's mask tiles are
            # DMA'd inside the front stream so its PART adds never stall
            def n_full(hf):
                return sum(c == FULL for kb in pattern[hf] for c in kb)

            hf_order = sorted(range(2), key=lambda hf: n_full(hf))
            part_blocks, slots, mask_tiles = {}, {}, {}
            for hf_i, hf in enumerate(hf_order):
                pb = [
                    (kb, nn)
                    for kb in range(16) for nn in range(2)
                    if pattern[hf][kb][nn] == PART
                ]
                part_blocks[hf] = pb
                slots[hf] = {blk: i for i, blk in enumerate(pb)}
                if len(pb) == 0:
                    mask_tiles[hf] = None
                elif len(pb) <= 8:
                    if hf_i == 0:
                        mask_tiles[hf] = maskp.tile(
                            [128, 8, 512], F32R, tag="mask", name=f"mask{hf}")
                    else:
                        mask_tiles[hf] = wqkp.tile(
                            [128, 8, 512], F32R, tag="wqkslot", name=f"mask{hf}")
                else:
                    mask_tiles[hf] = bigp.tile(
                        [128, 32, 512], F32R, tag="big", name=f"mask{hf}")

            def mask_dmas(hf, qs_):
                pb = part_blocks[hf]
                i = 0
                while i < len(pb):
                    kb0, nn0 = pb[i]
                    j = i + 1
                    while (j < len(pb)
                           and pb[j] == (pb[j - 1][0] + 1, nn0)):
                        j += 1
                    n = j - i
                    nc.sync.dma_start(
                        mask_tiles[hf][:, i : i + n, :],
                        maskT[ds(kb0 * 128, n * 128),
                              ds(qs_ + nn0 * 512, 512)].rearrange(
                            "(b p) q -> p b q", p=128),
                    )
                    i = j
            hf_first = hf_order[0]
            if mask_tiles.get(hf_first) is not None and len(part_blocks[hf_first]) <= 8:
                mask_dmas(hf_first, hf_first * 1024)
            # second s-half of x^T + wo
            for dc in range(8):
                nc.sync.dma_start(
                    xt_sb[:, dc, ds(1024, 1024)],
                    xT[ds(dc * 128, 128), ds(1024, 1024)],
                )
            wo_sb = wop.tile([128, 2, D], F32R)
            nc.sync.dma_start(wo_sb[:], wo[:].rearrange("(kc p) d -> p kc d", p=128))

            # ---- projections: QK chunks 0,2 first (unblocks heads 0/1),
            # then V, then QK chunks 1,3 (heads 2/3). qkT[c,s] = W^T x^T.
            def qk_chunk_half(pool, cc, sh):
                ps = pool.tile([128, 1024], F32, tag="psqk", name=f"qkps{cc}{sh}")
                for dc in range(8):
                    lhsT = wqk_sb[:, dc, ts(cc, 128)]
                    for nn in range(2):
                        nc.tensor.matmul(
                            ps[:, ts(nn, 512)], lhsT,
                            xt_sb[:, dc, ds(sh * 1024 + nn * 512, 512)],
                            start=(dc == 0), stop=(dc == 7),
                        )
                nc.scalar.activation(
                    qk_sb[:, cc, ds(sh * 1024, 1024)], ps[:], AF.Identity,
                    bias=b_sb[:, cc : cc + 1],
                )

            def v_blocks(pool, rng):
                for sb_i in rng:
                    ps = pool.tile([128, 256], F32, tag="psv", name=f"vps{sb_i}")
                    for dc in range(8):
                        nc.tensor.matmul(
                            ps[:], xt_sb[:, dc, ts(sb_i, 128)], wv_sb[:, dc, :],
                            start=(dc == 0), stop=False,
                        )
                    # += ones[s] * bv  (rank-1 bias add)
                    nc.tensor.matmul(ps[:], ones_t[:], bv_sb[:], start=False, stop=True)
                    nc.vector.tensor_copy(
                        v_sb[:, sb_i, :, 0:64],
                        ps[:].rearrange("p (h e) -> p h e", h=HPC),
                    )

            # half-0 projections first (q-half-0 attention unblocks early),
            # then half-1
            with (
                tc.tile_pool(name="psqk", bufs=2, space="PSUM") as psqk,
                tc.tile_pool(name="psv", bufs=2, space="PSUM") as psv,
            ):
                for cc in (0, 2, 1, 3):
                    qk_chunk_half(psqk, cc, 0)
                v_blocks(psv, range(8))
                for cc in (0, 2, 1, 3):
                    qk_chunk_half(psqk, cc, 1)
                v_blocks(psv, range(8, 16))

            # ---- attention, in [k, q] layout, q processed in two halves ----
            # (mask tiles + first-half mask DMAs hoisted into the front stream)
            with (
                tc.tile_pool(name="pssc", bufs=3, space="PSUM") as pssc,
                tc.tile_pool(name="psav", bufs=1, space="PSUM") as psav,
            ):
                def outproj(vals_t, qs_):
                    for ob in range(8):
                        ps = pssc.tile([128, 1024], F32, tag="pssc")
                        for kc in range(2):
                            lhsT = wo_sb[:, kc, ts(ob, 128)]
                            for nn in range(2):
                                nc.tensor.matmul(
                                    ps[:, ts(nn, 512)], lhsT,
                                    vals_t[:, kc, ts(nn, 512)],
                                    start=(kc == 0), stop=(kc == 1),
                                )
                        oev = attnp.tile([128, 1024], F32, tag="attn")
                        nc.scalar.activation(oev[:], ps[:], AF.Copy)
                        nc.sync.dma_start(
                            outT[ds(ob * 128, 128), ds(qs_, 1024)], oev[:]
                        )

                pending = None
                for hf_i, hf in enumerate(hf_order):
                    qs = hf * 1024
                    slot = slots[hf]
                    mask_sb = mask_tiles[hf]
                    if mask_sb is not None and (
                        hf_i > 0 or len(part_blocks[hf]) > 8
                    ):
                        mask_dmas(hf, qs)
                    vals_sb = valsp.tile(
                        [128, 2, 1024], F32R, tag="vals", name=f"vals{hf}")
                    kb_order = sorted(
                        (kb for kb in range(16) if pattern[hf][kb] != (SKIP, SKIP)),
                        key=lambda kb: (PART in pattern[hf][kb], kb),
                    )
                    for h in range(HPC):
                        off = 64 * (h % 2)
                        qt = qk_sb[off : off + 64, h // 2, :]
                        kt = qk_sb[off : off + 64, 2 + h // 2, :]
                        act_kbs = [
                            [kb for kb in kb_order if pattern[hf][kb][nn] != SKIP]
                            for nn in range(2)
                        ]
                        ps_av = psav.tile([65, 1024], F32, tag="psav")
                        for kb in kb_order:
                            cls = pattern[hf][kb]
                            ps_sc = pssc.tile([128, 1024], F32, tag="pssc")
                            lhsT = kt[:, ts(kb, 128)]
                            at = attnp.tile([128, 1024], F32R, tag="attn")
                            for nn in range(2):
                                if cls[nn] == SKIP:
                                    continue
                                nc.tensor.matmul(
                                    ps_sc[:, ts(nn, 512)], lhsT,
                                    qt[:, ds(qs + nn * 512, 512)],
                                    start=True, stop=True,
                                )
                            if cls == (FULL, FULL):
                                nc.scalar.activation(at[:], ps_sc[:], AF.Exp)
                            else:
                                for nn in range(2):
                                    if cls[nn] == SKIP:
                                        continue
                                    if cls[nn] == PART:
                                        nc.vector.tensor_tensor(
                                            at[:, ts(nn, 512)], ps_sc[:, ts(nn, 512)],
                                            mask_sb[:, slot[(kb, nn)], :], ADD,
                                        )
                                        nc.scalar.activation(
                                            at[:, ts(nn, 512)], at[:, ts(nn, 512)],
                                            AF.Exp,
                                        )
                                    else:
                                        nc.scalar.activation(
                                            at[:, ts(nn, 512)], ps_sc[:, ts(nn, 512)],
                                            AF.Exp,
                                        )
                            for nn in range(2):
                                if cls[nn] == SKIP:
                                    continue
                                nc.tensor.matmul(
                                    ps_av[:, ts(nn, 512)], v_sb[:, kb, h, :],
                                    at[:, ts(nn, 512)],
                                    start=(kb == act_kbs[nn][0]),
                                    stop=(kb == act_kbs[nn][-1]),
                                )
                        # normalize: vals = av[0:64] * (1 / av[64])
                        recip = smallp.tile([1, 1024], F32R, tag="recip")
                        with nc.allow_low_precision(
                            reason="float32r has fp32 bits; only PE matmul mode differs"
                        ):
                            nc.vector.reciprocal(recip[:], ps_av[64:65, :])
                        bc_sb = smallp.tile([64, 1024], F32R, tag="bc")
                        nc.gpsimd.partition_broadcast(bc_sb[:], recip[:])
                        nc.vector.tensor_tensor(
                            vals_sb[off : off + 64, h // 2, :],
                            ps_av[0:64, :], bc_sb[:], MULT,
                        )
                        if pending is not None and hf_i == 1 and h == 0:
                            outproj(*pending)
                            pending = None

                    # partial output projection for this q-half; the first
                    # half's is deferred into the second half's first head
                    # sweep so it fills gaps instead of competing at the
                    # transition (execution follows deps, not emission order)
                    if hf_i == 0 and len(hf_order) > 1:
                        pending = (vals_sb, qs)
                    else:
                        outproj(vals_sb, qs)
                if pending is not None:
                    outproj(*pending)
                    pending = None

    nc.compile()
    return nc


def _prep_inputs(x, mask, W_qkv, b_qkv, W_o, b_o):
    """Host-side sharding/layout prep: slices, transposes, 1/sqrt(HD) folding."""
    scale = np.float32(1.0 / np.sqrt(HD))
    xT = [np.ascontiguousarray(x[b].T) for b in range(B)]
    maskT = np.ascontiguousarray(mask.T)
    in_maps = []
    for c in range(NCORES):
        b, g = divmod(c, HPC)
        heads = [HPC * g + i for i in range(HPC)]
        qcols = np.concatenate(
            [W_qkv[:, 192 * h : 192 * h + 64] for h in heads], axis=1) * scale
        kcols = np.concatenate(
            [W_qkv[:, 192 * h + 64 : 192 * h + 128] for h in heads], axis=1)
        wqk = np.ascontiguousarray(np.concatenate([qcols, kcols], axis=1))
        bq = np.concatenate([b_qkv[192 * h : 192 * h + 64] for h in heads]) * scale
        bk = np.concatenate([b_qkv[192 * h + 64 : 192 * h + 128] for h in heads])
        bqk_t = np.ascontiguousarray(
            np.concatenate([bq, bk]).reshape(4, 128).T)
        wv = np.ascontiguousarray(np.concatenate(
            [W_qkv[:, 192 * h + 128 : 192 * h + 192] for h in heads], axis=1))
        bv = np.ascontiguousarray(np.concatenate(
            [b_qkv[192 * h + 128 : 192 * h + 192] for h in heads])[None, :])
        wo = np.ascontiguousarray(W_o[256 * g : 256 * (g + 1), :])
        in_maps.append({
            "xT": xT[b], "wqk": wqk, "bqk": bqk_t, "wv": wv, "bv": bv,
            "wo": wo, "maskT": maskT,
        })
    return in_maps


def kernel(x, mask, W_qkv, b_qkv, W_o, b_o):
    x = np.asarray(x, dtype=np.float32)
    mask = np.asarray(mask, dtype=np.float32)
    W_qkv = np.asarray(W_qkv, dtype=np.float32)
    b_qkv = np.asarray(b_qkv, dtype=np.float32)
    W_o = np.asarray(W_o, dtype=np.float32)
    b_o = np.asarray(b_o, dtype=np.float32)

    pattern = _classify_mask(np.ascontiguousarray(mask.T))
    key = ("nc", pattern)
    if key not in _CACHE:
        _CACHE[key] = _build(pattern)
    nc = _CACHE[key]
    _CACHE["nc"] = nc

    in_maps = _prep_inputs(x, mask, W_qkv, b_qkv, W_o, b_o)
    res = run_bass_kernel_spmd(nc, in_maps, core_ids=list(range(NCORES)))
    _CACHE["last_result"] = res

    out = np.empty((B, S, D), dtype=np.float32)
    for b in range(B):
        acc = res.results[HPC * b]["outT"].astype(np.float32)
        for g in range(1, HPC):
            acc = acc + res.results[HPC * b + g]["outT"]
        out[b] = acc.T + b_o
    return out
